# revision 15
# baseline (speedup 1.0000x reference)
"""VQ codebook bottleneck block (eval forward) on 8 Trainium2 NeuronCores.

Data-parallel: the flattened token dim (N*T = 65536 tokens) is sharded across
8 cores; since T == 2048 and each batch row of x is a contiguous (width, T)
slab, each core takes 4 full batches of x.  The codebook is replicated.

Per-core device program (all fp32 — argmin near-ties require fp32-grade
matmul precision; bf16 flips hundreds of indices):
  score[t, j] = 2*<x_t, k_j> - ||k_j||^2         (PE matmul, K=512 in 4 chunks,
                                                  argmin dist == argmax score)
  max/argmax over 2048 bins                       (DVE tensor_tensor_reduce +
                                                  max_index)
  x_d gather: k[argmax] per token                 (GPSIMD indirect DMA)
  transpose gathered rows to (width, T) layout    (PE transpose via identity)
  partial sums for prenorm/fit/commit_loss        (DVE reductions + PE ones-
                                                  matmul partition reduce)
Host combines per-core partials in fp64.
"""

import numpy as np

_CACHE = {}

# debug scoping: "full" | "noxd" (skip gather/transpose/xd) | "nomax"
# (skip max_index/gather/xd/xl) | "mm" (matmuls+stats only)
SCOPE = "full"

# "f32": native fp32 matmuls (4 cycles/row, max precision)
# "split3": bf16 hi/lo decomposition, 3 bf16 matmuls (3 cycles/row + fast
#           weight loads); ~2e-4 score error vs fp32's ~3e-5
DOT_MODE = "f32"

N_CORES = 8
NB = 4            # batches per core (32 / 8)
W = 512           # emb width
T = 2048          # tokens per batch
KB = 2048         # codebook bins
P = 128
WC = W // P       # 4 width chunks (contraction)
TT = T // P       # 16 token tiles per batch
NTOK = NB * T     # tokens per core


def _build_program():
    import concourse.bass as bass
    import concourse.mybir as mybir
    import concourse.tile as tile
    from concourse import bacc
    from concourse.masks import make_identity
    from contextlib import ExitStack

    f32 = mybir.dt.float32
    u32 = mybir.dt.uint32
    Alu = mybir.AluOpType
    Ax = mybir.AxisListType

    nc = bacc.Bacc(
        "TRN2",
        target_bir_lowering=False,
        debug=False,
        num_devices=N_CORES,
    )

    x_d = nc.declare_dram_parameter("x", [NB, W, T], f32, isOutput=False)
    kt2_d = nc.declare_dram_parameter("kt2", [W, KB], f32, isOutput=False)
    nk2_d = nc.declare_dram_parameter("nk2", [P, KB], f32, isOutput=False)
    kcb_d = nc.declare_dram_parameter("kcb", [KB, W], f32, isOutput=False)

    xd_d = nc.declare_dram_parameter("xd", [NB, W, T], f32, isOutput=True)
    xl_d = nc.declare_dram_parameter("xl", [NB, T], u32, isOutput=True)
    st_d = nc.declare_dram_parameter("st", [3, 1], f32, isOutput=True)

    FMIN = -3.4e38

    with tile.TileContext(nc) as tc, ExitStack() as ctx:
        const = ctx.enter_context(tc.tile_pool(name="const", bufs=1))
        xpool = ctx.enter_context(
            tc.tile_pool(name="xpool", bufs=2 if DOT_MODE == "f32" else 1)
        )
        xsplit = ctx.enter_context(tc.tile_pool(name="xsplit", bufs=2))
        spool = ctx.enter_context(tc.tile_pool(name="spool", bufs=2))
        gpool = ctx.enter_context(tc.tile_pool(name="gpool", bufs=3))
        xdpool = ctx.enter_context(tc.tile_pool(name="xdpool", bufs=3))
        small = ctx.enter_context(tc.tile_pool(name="small", bufs=3))
        accp = ctx.enter_context(tc.tile_pool(name="accp", bufs=1))
        scratch = ctx.enter_context(tc.tile_pool(name="scratch", bufs=1))
        pspool = ctx.enter_context(tc.tile_pool(name="pspool", bufs=3, space="PSUM"))
        tpool = ctx.enter_context(tc.tile_pool(name="tpool", bufs=2, space="PSUM"))

        # constants
        kt2_sb = const.tile([P, WC, KB], f32, tag="kt2")
        nc.sync.dma_start(kt2_sb[:], kt2_d.rearrange("(c p) j -> p c j", p=P))
        if DOT_MODE == "split3":
            bf16 = mybir.dt.bfloat16
            kthi = const.tile([P, WC, KB], bf16, tag="kthi")
            ktlo = const.tile([P, WC, KB], bf16, tag="ktlo")
            for c in range(WC):
                nc.vector.tensor_copy(kthi[:, c], kt2_sb[:, c])
                nc.vector.tensor_tensor(
                    out=ktlo[:, c], in0=kt2_sb[:, c], in1=kthi[:, c],
                    op=Alu.subtract,
                )
        nk2_sb = const.tile([P, KB], f32, tag="nk2")
        nc.sync.dma_start(nk2_sb[:], nk2_d[:])
        ident = const.tile([P, P], f32, tag="ident")
        make_identity(nc, ident[:])
        ones = const.tile([P, 1], f32, tag="ones")
        nc.vector.memset(ones[:], 1.0)

        # accumulators
        macc = accp.tile([P, NB * TT], f32, tag="macc")
        sxacc = accp.tile([P, NB * WC], f32, tag="sxacc")
        sx2acc = accp.tile([P, NB * WC], f32, tag="sx2acc")

        pending = None  # deferred gather->transpose->store stage (1-tile SW pipeline)

        def flush_pending():
            nonlocal pending
            if pending is None:
                return
            G, idx8, n_, t_ = pending
            tok_ = slice(t_ * P, (t_ + 1) * P)
            tps = tpool.tile([P, W], f32, tag="tp")
            for wb in range(WC):
                nc.tensor.transpose(
                    tps[:, wb * P:(wb + 1) * P], G[:, wb * P:(wb + 1) * P], ident[:]
                )
            xdt = xdpool.tile([P, W], f32, tag="xd")
            nc.scalar.copy(xdt[:], tps[:])
            nc.sync.dma_start(
                xd_d[n_, :, tok_].rearrange("(wb p) t -> p wb t", p=P),
                xdt[:].rearrange("p (wb t) -> p wb t", wb=WC),
            )
            nc.sync.dma_start(xl_d[n_, tok_], idx8[:, 0:1])
            pending = None

        for n in range(NB):
            x_sb = xpool.tile([P, WC, T], f32, tag="x")
            nc.sync.dma_start(x_sb[:], x_d[n].rearrange("(c p) t -> p c t", p=P))

            # element stats (prenorm / fit): sum x and sum x^2 per chunk.
            # Runs on the (otherwise mostly idle) ScalarEngine via
            # activation accum_out (= row-sum of the activated output).
            for c in range(WC):
                ch = n * WC + c
                scr = scratch.tile([P, T], f32, tag="scr")
                nc.scalar.activation(
                    out=scr[:], in_=x_sb[:, c],
                    func=mybir.ActivationFunctionType.Identity,
                    accum_out=sxacc[:, ch:ch + 1],
                )
                scr2 = scratch.tile([P, T], f32, tag="scr")
                nc.scalar.activation(
                    out=scr2[:], in_=x_sb[:, c],
                    func=mybir.ActivationFunctionType.Square,
                    accum_out=sx2acc[:, ch:ch + 1],
                )

            if DOT_MODE == "split3":
                bf16 = mybir.dt.bfloat16
                xhi = xsplit.tile([P, WC, T], bf16, tag="xhi")
                xlo = xsplit.tile([P, WC, T], bf16, tag="xlo")
                for c in range(WC):
                    nc.vector.tensor_copy(xhi[:, c], x_sb[:, c])
                    nc.vector.tensor_tensor(
                        out=xlo[:, c], in0=x_sb[:, c], in1=xhi[:, c],
                        op=Alu.subtract,
                    )

            for t in range(TT):
                ti = n * TT + t
                tok = slice(t * P, (t + 1) * P)
                score_sb = spool.tile([P, KB], f32, tag="score")
                for h in range(2):
                    ps = pspool.tile([P, 1024], f32, tag="ps")
                    # kc outer / bin-chunk inner: consecutive matmuls share the
                    # stationary operand (same per-element accumulation order)
                    if DOT_MODE == "f32":
                        for kc in range(WC):
                            for b2 in range(2):
                                b = h * 2 + b2
                                nc.tensor.matmul(
                                    ps[:, b2 * 512:(b2 + 1) * 512],
                                    lhsT=x_sb[:, kc, tok],
                                    rhs=kt2_sb[:, kc, b * 512:(b + 1) * 512],
                                    start=(kc == 0),
                                    stop=(kc == WC - 1),
                                )
                    else:
                        terms = ((xhi, kthi), (xhi, ktlo), (xlo, kthi))
                        for kc in range(WC):
                            for tm, (xa, ka) in enumerate(terms):
                                for b2 in range(2):
                                    b = h * 2 + b2
                                    nc.tensor.matmul(
                                        ps[:, b2 * 512:(b2 + 1) * 512],
                                        lhsT=xa[:, kc, tok],
                                        rhs=ka[:, kc, b * 512:(b + 1) * 512],
                                        start=(kc == 0 and tm == 0),
                                        stop=(kc == WC - 1 and tm == 2),
                                    )
                    # bias add (-||k||^2) while moving PSUM -> SBUF
                    nc.vector.tensor_tensor(
                        out=score_sb[:, h * 1024:(h + 1) * 1024],
                        in0=ps[:],
                        in1=nk2_sb[:, h * 1024:(h + 1) * 1024],
                        op=Alu.add,
                    )
                mx8 = small.tile([P, 8], f32, tag="mx8")
                nc.vector.max(out=mx8[:], in_=score_sb[:])
                # per-token max score into accumulator column (for fit)
                nc.vector.tensor_copy(macc[:, ti:ti + 1], mx8[:, 0:1])
                if SCOPE in ("nomax", "mm"):
                    continue
                idx8 = small.tile([P, 8], u32, tag="idx8")
                nc.vector.max_index(idx8[:], mx8[:], score_sb[:])

                if SCOPE == "noxd":
                    nc.sync.dma_start(xl_d[n, tok], idx8[:, 0:1])
                    continue

                # gather codebook rows for these 128 tokens
                G = gpool.tile([P, W], f32, tag="g")
                nc.gpsimd.indirect_dma_start(
                    out=G[:],
                    out_offset=None,
                    in_=kcb_d[:],
                    in_offset=bass.IndirectOffsetOnAxis(ap=idx8[:, 0:1], axis=0),
                )

                flush_pending()
                pending = (G, idx8, n, t)

        flush_pending()

        # final scalars: partition-reduce [sum_x, sum_x2, sum_maxscore] via matmul
        red3 = accp.tile([P, 3], f32, tag="red3")
        nc.vector.tensor_reduce(out=red3[:, 0:1], in_=sxacc[:], axis=Ax.X, op=Alu.add)
        nc.vector.tensor_reduce(out=red3[:, 1:2], in_=sx2acc[:], axis=Ax.X, op=Alu.add)
        nc.vector.tensor_reduce(out=red3[:, 2:3], in_=macc[:], axis=Ax.X, op=Alu.add)
        ps3 = tpool.tile([P, 1], f32, tag="tp")
        nc.tensor.matmul(ps3[:3, 0:1], lhsT=red3[:], rhs=ones[:], start=True, stop=True)
        st_sb = small.tile([P, 1], f32, tag="st")
        nc.scalar.copy(st_sb[:3], ps3[:3, 0:1])
        nc.sync.dma_start(st_d[:], st_sb[:3, :])

    nc.compile()
    return nc


def _get_program():
    key = (SCOPE, DOT_MODE)
    if key not in _CACHE:
        _CACHE[key] = _build_program()
    return _CACHE[key]


def kernel(x, k, _want_profile=False):
    from concourse.bass_utils import run_bass_kernel_spmd

    x = np.asarray(x)
    k = np.asarray(k)
    assert x.shape == (32, 512, 2048) and k.shape == (2048, 512)

    xf32 = np.ascontiguousarray(x, dtype=np.float32)
    kf32 = np.ascontiguousarray(k, dtype=np.float32)

    kt2 = np.ascontiguousarray((2.0 * kf32).T)                 # [512, 2048]
    nk2_row = -np.sum(kf32 * kf32, axis=-1)                    # [2048] fp32
    nk2 = np.ascontiguousarray(np.broadcast_to(nk2_row[None, :], (P, KB)))

    nc = _get_program()

    in_maps = []
    for c in range(N_CORES):
        in_maps.append({
            "x": np.ascontiguousarray(xf32[c * NB:(c + 1) * NB]),
            "kt2": kt2,
            "nk2": nk2,
            "kcb": kf32,
        })

    res = run_bass_kernel_spmd(nc, in_maps, core_ids=list(range(N_CORES)),
                               trace=_want_profile)

    xd = np.concatenate([np.asarray(r["xd"]) for r in res.results], axis=0)
    xl = np.concatenate([np.asarray(r["xl"]) for r in res.results], axis=0)
    xl = xl.astype(np.int32)

    sx = 0.0
    sx2 = 0.0
    smax = 0.0
    for r in res.results:
        st = np.asarray(r["st"], dtype=np.float64).reshape(-1)
        sx += st[0]
        sx2 += st[1]
        smax += st[2]

    n_elem = float(32 * T * W)
    n_tok = float(32 * T)
    prenorm = np.float32(np.sqrt(max(sx2 - sx * sx / n_elem, 0.0) / n_elem))
    fit = np.float32((sx2 - smax) / n_tok)
    commit_loss = np.float32((sx2 - smax) / n_elem)

    if _want_profile:
        _CACHE["last_exec_time_ns"] = res.exec_time_ns
        _CACHE["last_mean_exec_time_ns"] = res.mean_exec_time_ns

    return (
        xl,
        xd,
        np.array(commit_loss, dtype=np.float32),
        np.array(fit, dtype=np.float32),
        np.array(prenorm, dtype=np.float32),
    )


# revision 19
# speedup vs baseline: 1.6074x; 1.6074x over previous
"""VQ codebook bottleneck block (eval forward) on 8 Trainium2 NeuronCores.

Data-parallel: the flattened token dim (N*T = 65536 tokens) is sharded across
8 cores; since T == 2048 and each batch row of x is a contiguous (width, T)
slab, each core takes 4 full batches of x.  The codebook is replicated.

Per-core device program (all fp32 — argmin near-ties require fp32-grade
matmul precision; bf16 flips hundreds of indices):
  score[t, j] = 2*<x_t, k_j> - ||k_j||^2         (PE matmul, K=512 in 4 chunks,
                                                  argmin dist == argmax score)
  max/argmax over 2048 bins                       (DVE tensor_tensor_reduce +
                                                  max_index)
  x_d gather: k[argmax] per token                 (GPSIMD indirect DMA)
  transpose gathered rows to (width, T) layout    (PE transpose via identity)
  partial sums for prenorm/fit/commit_loss        (DVE reductions + PE ones-
                                                  matmul partition reduce)
Host combines per-core partials in fp64.
"""

import numpy as np

_CACHE = {}

# debug scoping: "full" | "noxd" (skip gather/transpose/xd) | "nomax"
# (skip max_index/gather/xd/xl) | "mm" (matmuls+stats only)
SCOPE = "full"

# "f32": native fp32 matmuls (4 cycles/row, max precision)
# "split3": bf16 hi/lo decomposition, 3 bf16 matmuls (3 cycles/row + fast
#           weight loads); ~2e-4 score error vs fp32's ~3e-5
DOT_MODE = "f32"

# matmul emission order: "kc_inner" (bin-chunk outer) or "kc_outer"
# (consecutive matmuls share the stationary operand). Measured on HW:
# kc_inner 0.81ms vs kc_outer 1.62ms — interleaving accumulation regions
# degrades the fp32 hi/lo matmul pipeline badly.
MM_ORDER = "kc_inner"

N_CORES = 8
NB = 4            # batches per core (32 / 8)
W = 512           # emb width
T = 2048          # tokens per batch
KB = 2048         # codebook bins
P = 128
WC = W // P       # 4 width chunks (contraction)
TT = T // P       # 16 token tiles per batch
NTOK = NB * T     # tokens per core


def _build_program():
    import concourse.bass as bass
    import concourse.mybir as mybir
    import concourse.tile as tile
    from concourse import bacc
    from concourse.masks import make_identity
    from contextlib import ExitStack

    f32 = mybir.dt.float32
    u32 = mybir.dt.uint32
    Alu = mybir.AluOpType
    Ax = mybir.AxisListType

    nc = bacc.Bacc(
        "TRN2",
        target_bir_lowering=False,
        debug=False,
        num_devices=N_CORES,
    )

    x_d = nc.declare_dram_parameter("x", [NB, W, T], f32, isOutput=False)
    kt2_d = nc.declare_dram_parameter("kt2", [W, KB], f32, isOutput=False)
    nk2_d = nc.declare_dram_parameter("nk2", [P, KB], f32, isOutput=False)
    kcb_d = nc.declare_dram_parameter("kcb", [KB, W], f32, isOutput=False)

    xd_d = nc.declare_dram_parameter("xd", [NB, W, T], f32, isOutput=True)
    xl_d = nc.declare_dram_parameter("xl", [NB, T], u32, isOutput=True)
    st_d = nc.declare_dram_parameter("st", [3, 1], f32, isOutput=True)

    FMIN = -3.4e38

    with tile.TileContext(nc) as tc, ExitStack() as ctx:
        const = ctx.enter_context(tc.tile_pool(name="const", bufs=1))
        xpool = ctx.enter_context(
            tc.tile_pool(name="xpool", bufs=2 if DOT_MODE == "f32" else 1)
        )
        xsplit = ctx.enter_context(tc.tile_pool(name="xsplit", bufs=2))
        spool = ctx.enter_context(tc.tile_pool(name="spool", bufs=2))
        gpool = ctx.enter_context(tc.tile_pool(name="gpool", bufs=3))
        xdpool = ctx.enter_context(tc.tile_pool(name="xdpool", bufs=3))
        small = ctx.enter_context(tc.tile_pool(name="small", bufs=3))
        accp = ctx.enter_context(tc.tile_pool(name="accp", bufs=1))
        scratch = ctx.enter_context(tc.tile_pool(name="scratch", bufs=1))
        pspool = ctx.enter_context(tc.tile_pool(name="pspool", bufs=3, space="PSUM"))
        tpool = ctx.enter_context(tc.tile_pool(name="tpool", bufs=2, space="PSUM"))

        # constants
        kt2_sb = const.tile([P, WC, KB], f32, tag="kt2")
        nc.sync.dma_start(kt2_sb[:], kt2_d.rearrange("(c p) j -> p c j", p=P))
        if DOT_MODE == "split3":
            bf16 = mybir.dt.bfloat16
            kthi = const.tile([P, WC, KB], bf16, tag="kthi")
            ktlo = const.tile([P, WC, KB], bf16, tag="ktlo")
            for c in range(WC):
                nc.vector.tensor_copy(kthi[:, c], kt2_sb[:, c])
                nc.vector.tensor_tensor(
                    out=ktlo[:, c], in0=kt2_sb[:, c], in1=kthi[:, c],
                    op=Alu.subtract,
                )
        nk2_sb = const.tile([P, KB], f32, tag="nk2")
        nc.sync.dma_start(nk2_sb[:], nk2_d[:])
        ident = const.tile([P, P], f32, tag="ident")
        make_identity(nc, ident[:])
        ones = const.tile([P, 1], f32, tag="ones")
        nc.vector.memset(ones[:], 1.0)

        # accumulators
        macc = accp.tile([P, NB * TT], f32, tag="macc")
        sxacc = accp.tile([P, NB * WC], f32, tag="sxacc")
        sx2acc = accp.tile([P, NB * WC], f32, tag="sx2acc")

        pending = None  # deferred gather->transpose->store stage (1-tile SW pipeline)

        def flush_pending():
            nonlocal pending
            if pending is None:
                return
            G, idx8, n_, t_ = pending
            tok_ = slice(t_ * P, (t_ + 1) * P)
            tps = tpool.tile([P, W], f32, tag="tp")
            for wb in range(WC):
                nc.tensor.transpose(
                    tps[:, wb * P:(wb + 1) * P], G[:, wb * P:(wb + 1) * P], ident[:]
                )
            xdt = xdpool.tile([P, W], f32, tag="xd")
            nc.scalar.copy(xdt[:], tps[:])
            nc.sync.dma_start(
                xd_d[n_, :, tok_].rearrange("(wb p) t -> p wb t", p=P),
                xdt[:].rearrange("p (wb t) -> p wb t", wb=WC),
            )
            nc.sync.dma_start(xl_d[n_, tok_], idx8[:, 0:1])
            pending = None

        for n in range(NB):
            x_sb = xpool.tile([P, WC, T], f32, tag="x")
            nc.sync.dma_start(x_sb[:], x_d[n].rearrange("(c p) t -> p c t", p=P))

            # element stats (prenorm / fit): sum x and sum x^2 per chunk.
            # Runs on the (otherwise mostly idle) ScalarEngine via
            # activation accum_out (= row-sum of the activated output).
            for c in range(WC):
                ch = n * WC + c
                scr = scratch.tile([P, T], f32, tag="scr")
                nc.scalar.activation(
                    out=scr[:], in_=x_sb[:, c],
                    func=mybir.ActivationFunctionType.Identity,
                    accum_out=sxacc[:, ch:ch + 1],
                )
                scr2 = scratch.tile([P, T], f32, tag="scr")
                nc.scalar.activation(
                    out=scr2[:], in_=x_sb[:, c],
                    func=mybir.ActivationFunctionType.Square,
                    accum_out=sx2acc[:, ch:ch + 1],
                )

            if DOT_MODE == "split3":
                bf16 = mybir.dt.bfloat16
                xhi = xsplit.tile([P, WC, T], bf16, tag="xhi")
                xlo = xsplit.tile([P, WC, T], bf16, tag="xlo")
                for c in range(WC):
                    nc.vector.tensor_copy(xhi[:, c], x_sb[:, c])
                    nc.vector.tensor_tensor(
                        out=xlo[:, c], in0=x_sb[:, c], in1=xhi[:, c],
                        op=Alu.subtract,
                    )

            for t in range(TT):
                ti = n * TT + t
                tok = slice(t * P, (t + 1) * P)
                score_sb = spool.tile([P, KB], f32, tag="score")
                for h in range(2):
                    ps = pspool.tile([P, 1024], f32, tag="ps")
                    # kc outer / bin-chunk inner: consecutive matmuls share the
                    # stationary operand (same per-element accumulation order)
                    if DOT_MODE == "f32":
                        if MM_ORDER == "kc_outer":
                            loop = [(kc, b2) for kc in range(WC) for b2 in range(2)]
                        else:
                            loop = [(kc, b2) for b2 in range(2) for kc in range(WC)]
                        for kc, b2 in loop:
                            b = h * 2 + b2
                            nc.tensor.matmul(
                                ps[:, b2 * 512:(b2 + 1) * 512],
                                lhsT=x_sb[:, kc, tok],
                                rhs=kt2_sb[:, kc, b * 512:(b + 1) * 512],
                                start=(kc == 0),
                                stop=(kc == WC - 1),
                            )
                    else:
                        terms = ((xhi, kthi), (xhi, ktlo), (xlo, kthi))
                        for kc in range(WC):
                            for tm, (xa, ka) in enumerate(terms):
                                for b2 in range(2):
                                    b = h * 2 + b2
                                    nc.tensor.matmul(
                                        ps[:, b2 * 512:(b2 + 1) * 512],
                                        lhsT=xa[:, kc, tok],
                                        rhs=ka[:, kc, b * 512:(b + 1) * 512],
                                        start=(kc == 0 and tm == 0),
                                        stop=(kc == WC - 1 and tm == 2),
                                    )
                    # bias add (-||k||^2) while moving PSUM -> SBUF
                    nc.vector.tensor_tensor(
                        out=score_sb[:, h * 1024:(h + 1) * 1024],
                        in0=ps[:],
                        in1=nk2_sb[:, h * 1024:(h + 1) * 1024],
                        op=Alu.add,
                    )
                mx8 = small.tile([P, 8], f32, tag="mx8")
                nc.vector.max(out=mx8[:], in_=score_sb[:])
                # per-token max score into accumulator column (for fit)
                nc.vector.tensor_copy(macc[:, ti:ti + 1], mx8[:, 0:1])
                if SCOPE in ("nomax", "mm"):
                    continue
                idx8 = small.tile([P, 8], u32, tag="idx8")
                nc.vector.max_index(idx8[:], mx8[:], score_sb[:])

                if SCOPE == "noxd":
                    nc.sync.dma_start(xl_d[n, tok], idx8[:, 0:1])
                    continue

                # gather codebook rows for these 128 tokens
                G = gpool.tile([P, W], f32, tag="g")
                nc.gpsimd.indirect_dma_start(
                    out=G[:],
                    out_offset=None,
                    in_=kcb_d[:],
                    in_offset=bass.IndirectOffsetOnAxis(ap=idx8[:, 0:1], axis=0),
                )

                flush_pending()
                pending = (G, idx8, n, t)

        flush_pending()

        # final scalars: partition-reduce [sum_x, sum_x2, sum_maxscore] via matmul
        red3 = accp.tile([P, 3], f32, tag="red3")
        nc.vector.tensor_reduce(out=red3[:, 0:1], in_=sxacc[:], axis=Ax.X, op=Alu.add)
        nc.vector.tensor_reduce(out=red3[:, 1:2], in_=sx2acc[:], axis=Ax.X, op=Alu.add)
        nc.vector.tensor_reduce(out=red3[:, 2:3], in_=macc[:], axis=Ax.X, op=Alu.add)
        ps3 = tpool.tile([P, 1], f32, tag="tp")
        nc.tensor.matmul(ps3[:3, 0:1], lhsT=red3[:], rhs=ones[:], start=True, stop=True)
        st_sb = small.tile([P, 1], f32, tag="st")
        nc.scalar.copy(st_sb[:3], ps3[:3, 0:1])
        nc.sync.dma_start(st_d[:], st_sb[:3, :])

    nc.compile()
    return nc


def _get_program():
    key = (SCOPE, DOT_MODE, MM_ORDER)
    if key not in _CACHE:
        _CACHE[key] = _build_program()
    return _CACHE[key]


def kernel(x, k, _want_profile=False):
    from concourse.bass_utils import run_bass_kernel_spmd

    x = np.asarray(x)
    k = np.asarray(k)
    assert x.shape == (32, 512, 2048) and k.shape == (2048, 512)

    xf32 = np.ascontiguousarray(x, dtype=np.float32)
    kf32 = np.ascontiguousarray(k, dtype=np.float32)

    kt2 = np.ascontiguousarray((2.0 * kf32).T)                 # [512, 2048]
    nk2_row = -np.sum(kf32 * kf32, axis=-1)                    # [2048] fp32
    nk2 = np.ascontiguousarray(np.broadcast_to(nk2_row[None, :], (P, KB)))

    nc = _get_program()

    in_maps = []
    for c in range(N_CORES):
        in_maps.append({
            "x": np.ascontiguousarray(xf32[c * NB:(c + 1) * NB]),
            "kt2": kt2,
            "nk2": nk2,
            "kcb": kf32,
        })

    res = run_bass_kernel_spmd(nc, in_maps, core_ids=list(range(N_CORES)),
                               trace=_want_profile)

    xd = np.concatenate([np.asarray(r["xd"]) for r in res.results], axis=0)
    xl = np.concatenate([np.asarray(r["xl"]) for r in res.results], axis=0)
    xl = xl.astype(np.int32)

    sx = 0.0
    sx2 = 0.0
    smax = 0.0
    for r in res.results:
        st = np.asarray(r["st"], dtype=np.float64).reshape(-1)
        sx += st[0]
        sx2 += st[1]
        smax += st[2]

    n_elem = float(32 * T * W)
    n_tok = float(32 * T)
    prenorm = np.float32(np.sqrt(max(sx2 - sx * sx / n_elem, 0.0) / n_elem))
    fit = np.float32((sx2 - smax) / n_tok)
    commit_loss = np.float32((sx2 - smax) / n_elem)

    if _want_profile:
        _CACHE["last_exec_time_ns"] = res.exec_time_ns
        _CACHE["last_mean_exec_time_ns"] = res.mean_exec_time_ns

    return (
        xl,
        xd,
        np.array(commit_loss, dtype=np.float32),
        np.array(fit, dtype=np.float32),
        np.array(prenorm, dtype=np.float32),
    )


# revision 21
# speedup vs baseline: 2.3873x; 1.4851x over previous
"""VQ codebook bottleneck block (eval forward) on 8 Trainium2 NeuronCores.

Data-parallel: the flattened token dim (N*T = 65536 tokens) is sharded across
8 cores; since T == 2048 and each batch row of x is a contiguous (width, T)
slab, each core takes 4 full batches of x.  The codebook is replicated.

Per-core device program (all fp32 — argmin near-ties require fp32-grade
matmul precision; bf16 flips hundreds of indices):
  score[t, j] = 2*<x_t, k_j> - ||k_j||^2         (PE matmul, K=512 in 4 chunks,
                                                  argmin dist == argmax score)
  max/argmax over 2048 bins                       (DVE tensor_tensor_reduce +
                                                  max_index)
  x_d gather: k[argmax] per token                 (GPSIMD indirect DMA)
  transpose gathered rows to (width, T) layout    (PE transpose via identity)
  partial sums for prenorm/fit/commit_loss        (DVE reductions + PE ones-
                                                  matmul partition reduce)
Host combines per-core partials in fp64.
"""

import numpy as np

_CACHE = {}

# debug scoping: "full" | "noxd" (skip gather/transpose/xd) | "nomax"
# (skip max_index/gather/xd/xl) | "mm" (matmuls+stats only)
SCOPE = "full"

# "f32": native fp32 matmuls (4 cycles/row, max precision)
# "split3": bf16 hi/lo decomposition, 3 bf16 matmuls (3 cycles/row + fast
#           weight loads); ~2e-4 score error vs fp32's ~3e-5
DOT_MODE = "f32"

# matmul emission order: "kc_inner" (bin-chunk outer) or "kc_outer"
# (consecutive matmuls share the stationary operand). Measured on HW:
# kc_inner 0.81ms vs kc_outer 1.62ms — interleaving accumulation regions
# degrades the fp32 hi/lo matmul pipeline badly.
MM_ORDER = "kc_inner"

N_CORES = 8
NB = 4            # batches per core (32 / 8)
W = 512           # emb width
T = 2048          # tokens per batch
KB = 2048         # codebook bins
P = 128
WC = W // P       # 4 width chunks (contraction)
TT = T // P       # 16 token tiles per batch
NTOK = NB * T     # tokens per core


def _build_program():
    import concourse.bass as bass
    import concourse.mybir as mybir
    import concourse.tile as tile
    from concourse import bacc
    from concourse.masks import make_identity
    from contextlib import ExitStack

    f32 = mybir.dt.float32
    u32 = mybir.dt.uint32
    Alu = mybir.AluOpType
    Ax = mybir.AxisListType

    nc = bacc.Bacc(
        "TRN2",
        target_bir_lowering=False,
        debug=False,
        num_devices=N_CORES,
    )

    x_d = nc.declare_dram_parameter("x", [NB, W, T], f32, isOutput=False)
    kt2_d = nc.declare_dram_parameter("kt2", [W, KB], f32, isOutput=False)
    nk2_d = nc.declare_dram_parameter("nk2", [P, KB], f32, isOutput=False)
    kcb_d = nc.declare_dram_parameter("kcb", [KB, W], f32, isOutput=False)

    xd_d = nc.declare_dram_parameter("xd", [NB, W, T], f32, isOutput=True)
    xl_d = nc.declare_dram_parameter("xl", [NB, T], u32, isOutput=True)
    st_d = nc.declare_dram_parameter("st", [3, 1], f32, isOutput=True)

    FMIN = -3.4e38

    with tile.TileContext(nc) as tc, ExitStack() as ctx:
        const = ctx.enter_context(tc.tile_pool(name="const", bufs=1))
        xpool = ctx.enter_context(
            tc.tile_pool(name="xpool", bufs=2 if DOT_MODE == "f32" else 1)
        )
        xsplit = ctx.enter_context(tc.tile_pool(name="xsplit", bufs=2))
        spool = ctx.enter_context(tc.tile_pool(name="spool", bufs=2))
        gpool = ctx.enter_context(tc.tile_pool(name="gpool", bufs=3))
        xdpool = ctx.enter_context(tc.tile_pool(name="xdpool", bufs=3))
        small = ctx.enter_context(tc.tile_pool(name="small", bufs=3))
        accp = ctx.enter_context(tc.tile_pool(name="accp", bufs=1))
        scratch = ctx.enter_context(tc.tile_pool(name="scratch", bufs=1))
        pspool = ctx.enter_context(tc.tile_pool(name="pspool", bufs=3, space="PSUM"))
        tpool = ctx.enter_context(tc.tile_pool(name="tpool", bufs=2, space="PSUM"))

        # constants
        kt2_sb = const.tile([P, WC, KB], f32, tag="kt2")
        nc.sync.dma_start(kt2_sb[:], kt2_d.rearrange("(c p) j -> p c j", p=P))
        if DOT_MODE == "split3":
            bf16 = mybir.dt.bfloat16
            kthi = const.tile([P, WC, KB], bf16, tag="kthi")
            ktlo = const.tile([P, WC, KB], bf16, tag="ktlo")
            for c in range(WC):
                nc.vector.tensor_copy(kthi[:, c], kt2_sb[:, c])
                nc.vector.tensor_tensor(
                    out=ktlo[:, c], in0=kt2_sb[:, c], in1=kthi[:, c],
                    op=Alu.subtract,
                )
        nk2_sb = const.tile([P, KB], f32, tag="nk2")
        nc.sync.dma_start(nk2_sb[:], nk2_d[:])
        ident = const.tile([P, P], f32, tag="ident")
        make_identity(nc, ident[:])
        ones = const.tile([P, 1], f32, tag="ones")
        nc.vector.memset(ones[:], 1.0)

        # accumulators
        macc = accp.tile([P, NB * TT], f32, tag="macc")
        sxacc = accp.tile([P, NB * WC], f32, tag="sxacc")
        sx2acc = accp.tile([P, NB * WC], f32, tag="sx2acc")

        pending = None  # deferred gather->transpose->store stage (1-tile SW pipeline)

        def flush_pending():
            nonlocal pending
            if pending is None:
                return
            G, idx8, n_, t_ = pending
            tok_ = slice(t_ * P, (t_ + 1) * P)
            tps = tpool.tile([P, W], f32, tag="tp")
            for wb in range(WC):
                nc.tensor.transpose(
                    tps[:, wb * P:(wb + 1) * P], G[:, wb * P:(wb + 1) * P], ident[:]
                )
            xdt = xdpool.tile([P, W], f32, tag="xd")
            nc.scalar.copy(xdt[:], tps[:])
            nc.sync.dma_start(
                xd_d[n_, :, tok_].rearrange("(wb p) t -> p wb t", p=P),
                xdt[:].rearrange("p (wb t) -> p wb t", wb=WC),
            )
            nc.sync.dma_start(xl_d[n_, tok_], idx8[:, 0:1])
            pending = None

        for n in range(NB):
            x_sb = xpool.tile([P, WC, T], f32, tag="x")
            nc.sync.dma_start(x_sb[:], x_d[n].rearrange("(c p) t -> p c t", p=P))

            # element stats (prenorm / fit): sum x and sum x^2 per chunk.
            # Runs on the (otherwise mostly idle) ScalarEngine via
            # activation accum_out (= row-sum of the activated output).
            for c in range(WC):
                ch = n * WC + c
                scr = scratch.tile([P, T], f32, tag="scr")
                nc.scalar.activation(
                    out=scr[:], in_=x_sb[:, c],
                    func=mybir.ActivationFunctionType.Identity,
                    accum_out=sxacc[:, ch:ch + 1],
                )
                scr2 = scratch.tile([P, T], f32, tag="scr")
                nc.scalar.activation(
                    out=scr2[:], in_=x_sb[:, c],
                    func=mybir.ActivationFunctionType.Square,
                    accum_out=sx2acc[:, ch:ch + 1],
                )

            if DOT_MODE == "split3":
                bf16 = mybir.dt.bfloat16
                xhi = xsplit.tile([P, WC, T], bf16, tag="xhi")
                xlo = xsplit.tile([P, WC, T], bf16, tag="xlo")
                for c in range(WC):
                    nc.vector.tensor_copy(xhi[:, c], x_sb[:, c])
                    nc.vector.tensor_tensor(
                        out=xlo[:, c], in0=x_sb[:, c], in1=xhi[:, c],
                        op=Alu.subtract,
                    )

            for t in range(TT):
                ti = n * TT + t
                tok = slice(t * P, (t + 1) * P)
                score_sb = spool.tile([P, KB], f32, tag="score")
                for h in range(2):
                    ps = pspool.tile([P, 1024], f32, tag="ps")
                    # kc outer / bin-chunk inner: consecutive matmuls share the
                    # stationary operand (same per-element accumulation order)
                    if DOT_MODE == "f32":
                        if MM_ORDER == "kc_outer":
                            loop = [(kc, b2) for kc in range(WC) for b2 in range(2)]
                        else:
                            loop = [(kc, b2) for b2 in range(2) for kc in range(WC)]
                        for kc, b2 in loop:
                            b = h * 2 + b2
                            nc.tensor.matmul(
                                ps[:, b2 * 512:(b2 + 1) * 512],
                                lhsT=x_sb[:, kc, tok],
                                rhs=kt2_sb[:, kc, b * 512:(b + 1) * 512],
                                start=(kc == 0),
                                stop=(kc == WC - 1),
                            )
                    else:
                        terms = ((xhi, kthi), (xhi, ktlo), (xlo, kthi))
                        for kc in range(WC):
                            for tm, (xa, ka) in enumerate(terms):
                                for b2 in range(2):
                                    b = h * 2 + b2
                                    nc.tensor.matmul(
                                        ps[:, b2 * 512:(b2 + 1) * 512],
                                        lhsT=xa[:, kc, tok],
                                        rhs=ka[:, kc, b * 512:(b + 1) * 512],
                                        start=(kc == 0 and tm == 0),
                                        stop=(kc == WC - 1 and tm == 2),
                                    )
                    # bias add (-||k||^2) while moving PSUM -> SBUF
                    nc.vector.tensor_tensor(
                        out=score_sb[:, h * 1024:(h + 1) * 1024],
                        in0=ps[:],
                        in1=nk2_sb[:, h * 1024:(h + 1) * 1024],
                        op=Alu.add,
                    )
                mx8 = small.tile([P, 8], f32, tag="mx8")
                nc.vector.max(out=mx8[:], in_=score_sb[:])
                # per-token max score into accumulator column (for fit)
                nc.vector.tensor_copy(macc[:, ti:ti + 1], mx8[:, 0:1])
                if SCOPE in ("nomax", "mm"):
                    continue
                idx8 = small.tile([P, 8], u32, tag="idx8")
                nc.vector.max_index(idx8[:], mx8[:], score_sb[:])

                if SCOPE == "noxd":
                    nc.sync.dma_start(xl_d[n, tok], idx8[:, 0:1])
                    continue

                # gather codebook rows for these 128 tokens
                G = gpool.tile([P, W], f32, tag="g")
                nc.gpsimd.indirect_dma_start(
                    out=G[:],
                    out_offset=None,
                    in_=kcb_d[:],
                    in_offset=bass.IndirectOffsetOnAxis(ap=idx8[:, 0:1], axis=0),
                )

                flush_pending()
                pending = (G, idx8, n, t)

        flush_pending()

        # final scalars: partition-reduce [sum_x, sum_x2, sum_maxscore] via matmul
        red3 = accp.tile([P, 3], f32, tag="red3")
        nc.vector.tensor_reduce(out=red3[:, 0:1], in_=sxacc[:], axis=Ax.X, op=Alu.add)
        nc.vector.tensor_reduce(out=red3[:, 1:2], in_=sx2acc[:], axis=Ax.X, op=Alu.add)
        nc.vector.tensor_reduce(out=red3[:, 2:3], in_=macc[:], axis=Ax.X, op=Alu.add)
        ps3 = tpool.tile([P, 1], f32, tag="tp")
        nc.tensor.matmul(ps3[:3, 0:1], lhsT=red3[:], rhs=ones[:], start=True, stop=True)
        st_sb = small.tile([P, 1], f32, tag="st")
        nc.scalar.copy(st_sb[:3], ps3[:3, 0:1])
        nc.sync.dma_start(st_d[:], st_sb[:3, :])

    nc.compile()
    return nc


def _get_program():
    key = (SCOPE, DOT_MODE, MM_ORDER)
    if key not in _CACHE:
        _CACHE[key] = _build_program()
    return _CACHE[key]


def _get_runner():
    """Build (once) a jitted 8-way-sharded executable for the program.

    Mirrors concourse.bass2jax.run_bass_via_pjrt's multi-core path, but keeps
    the jitted callable so repeat kernel() calls skip re-lowering.
    """
    import jax
    import jax.numpy as jnp
    from jax.sharding import Mesh, PartitionSpec, NamedSharding
    from jax.experimental.shard_map import shard_map
    import concourse.mybir as mybir
    from concourse import bass2jax
    from concourse.bass2jax import _bass_exec_p, install_neuronx_cc_hook

    key = ("runner", SCOPE, DOT_MODE, MM_ORDER)
    if key in _CACHE:
        return _CACHE[key]

    nc = _get_program()
    install_neuronx_cc_hook()
    partition_name = nc.partition_id_tensor.name if nc.partition_id_tensor else None

    in_names, out_names, out_avals, out_shapes = [], [], [], []
    for alloc in nc.m.functions[0].allocations:
        if not isinstance(alloc, mybir.MemoryLocationSet):
            continue
        name = alloc.memorylocations[0].name
        if alloc.kind == "ExternalInput":
            if name != partition_name:
                in_names.append(name)
        elif alloc.kind == "ExternalOutput":
            shape = tuple(alloc.tensor_shape)
            dtype = mybir.dt.np(alloc.dtype)
            out_names.append(name)
            out_avals.append(jax.core.ShapedArray(shape, dtype))
            out_shapes.append((shape, dtype))
    n_params = len(in_names)
    n_outs = len(out_avals)
    all_in_names = list(in_names) + list(out_names)
    if partition_name is not None:
        all_in_names.append(partition_name)
    donate = tuple(range(n_params, n_params + n_outs))

    def _body(*args):
        operands = list(args)
        if partition_name is not None:
            operands.append(bass2jax.partition_id_tensor())
        outs = _bass_exec_p.bind(
            *operands,
            out_avals=tuple(out_avals),
            in_names=tuple(all_in_names),
            out_names=tuple(out_names),
            lowering_input_output_aliases=(),
            sim_require_finite=True,
            sim_require_nnan=True,
            nc=nc,
        )
        return tuple(outs)

    devices = jax.devices()[:N_CORES]
    mesh = Mesh(np.asarray(devices), ("core",))
    sharded = jax.jit(
        shard_map(
            _body, mesh=mesh,
            in_specs=(PartitionSpec("core"),) * (n_params + n_outs),
            out_specs=(PartitionSpec("core"),) * n_outs,
            check_rep=False,
        ),
        donate_argnums=donate,
        keep_unused=True,
    )
    sharding = NamedSharding(mesh, PartitionSpec("core"))

    def run(in_maps):
        concat_in = [
            np.concatenate([np.asarray(in_maps[c][nm]) for c in range(N_CORES)],
                           axis=0)
            for nm in in_names
        ]
        dev_in = [jax.device_put(a, sharding) for a in concat_in]
        zeros = [
            jax.device_put(jnp.zeros((N_CORES * s[0], *s[1:]), d), sharding)
            for (s, d) in out_shapes
        ]
        outs = sharded(*dev_in, *zeros)
        return [
            {nm: np.asarray(outs[i]).reshape(N_CORES, *out_shapes[i][0])[c]
             for i, nm in enumerate(out_names)}
            for c in range(N_CORES)
        ]

    _CACHE[key] = run
    return run


def kernel(x, k):
    x = np.asarray(x)
    k = np.asarray(k)
    assert x.shape == (32, 512, 2048) and k.shape == (2048, 512)

    xf32 = np.ascontiguousarray(x, dtype=np.float32)
    kf32 = np.ascontiguousarray(k, dtype=np.float32)

    kt2 = np.ascontiguousarray((2.0 * kf32).T)                 # [512, 2048]
    nk2_row = -np.sum(kf32 * kf32, axis=-1)                    # [2048] fp32
    nk2 = np.ascontiguousarray(np.broadcast_to(nk2_row[None, :], (P, KB)))

    in_maps = []
    for c in range(N_CORES):
        in_maps.append({
            "x": np.ascontiguousarray(xf32[c * NB:(c + 1) * NB]),
            "kt2": kt2,
            "nk2": nk2,
            "kcb": kf32,
        })

    results = _get_runner()(in_maps)

    xd = np.concatenate([np.asarray(r["xd"]) for r in results], axis=0)
    xl = np.concatenate([np.asarray(r["xl"]) for r in results], axis=0)
    xl = xl.astype(np.int32)

    sx = 0.0
    sx2 = 0.0
    smax = 0.0
    for r in results:
        st = np.asarray(r["st"], dtype=np.float64).reshape(-1)
        sx += st[0]
        sx2 += st[1]
        smax += st[2]

    n_elem = float(32 * T * W)
    n_tok = float(32 * T)
    prenorm = np.float32(np.sqrt(max(sx2 - sx * sx / n_elem, 0.0) / n_elem))
    fit = np.float32((sx2 - smax) / n_tok)
    commit_loss = np.float32((sx2 - smax) / n_elem)

    return (
        xl,
        xd,
        np.array(commit_loss, dtype=np.float32),
        np.array(fit, dtype=np.float32),
        np.array(prenorm, dtype=np.float32),
    )
